# revision 38
# baseline (speedup 1.0000x reference)
"""Adaptive downsample (CARAFE-like) Trainium2 kernel, 8-core data parallel.

Reference computation (for shapes [4, 96, 256, 512] f32):
  y    = conv3x3_s2_p1(x, w1) * (gamma/sqrt(1+eps)) + beta        # [b,192,128,256]
  y    = leaky_relu(y, 0.1)
  mask = conv1x1(y, w2)                                           # [b,144,128,256]
  mask = softmax over the 9 taps within each of 16 groups
  out[c] = sum_t mask[g(c), t] * unfold(x)[c, t]                  # [b,96,128,256]

Distribution: 8 shards = (batch 4) x (output-H halves 2); halo rows are
sliced host-side so there is no inter-core communication.

Layout:
 - channels permuted g-major (c' = 6g + i, original c = 16i + g) so the
   mask broadcast group->channels is one replication DMA with AP
   [[src_partition,16],[0,6],[1,N]]
 - x deinterleaved host-side per row into [O(258 incl pad) | E(256)]
   sections (O = [0, odd cols], E = even cols) so all conv taps and patch
   reads are contiguous slices; the kj=2 taps sit at +1 (odd element
   offset), so their element-wise products run on gpsimd (rp0) or at DVE
   1x mode (rp1) while kj=0/1 products keep the DVE 2x fast path
 - compute in bf16 (matmuls accumulate f32 in PSUM)

Schedule: 16 steps of 4 output rows (2 row-pairs) each, SOFTWARE-
PIPELINED with the element-wise phase lagging the mask phase by TWO
steps, so the group->channel replication DMAs have more than a full
step to execute before anything consumes them (engines execute their
streams in order; a wait at a queue head serializes the machine):
  step s emits:
    store(s-3)   output DMA (scalar)    - tree long since done, no wait
    loadx(s+2)   x slab prefetch (sync, ahead of the replication DMAs)
    front(s)     conv1/lrelu/conv2/exp/denominators (PE+scalar),
                 tap replication DMAs (sync)
    back(s-2)    patch products + tap-sum tree + normalize (DVE + one
                 gpsimd product)
    recip(s)     1/denominator (DVE) + recip replication (sync)
This keeps PE streaming back-to-back (HAM stays at 2.4 GHz) and gives
every DMA a step of slack before its consumer.
"""
import numpy as np
import ml_dtypes

BF = ml_dtypes.bfloat16

B, C, H, W = 4, 96, 256, 512
G, NI = 16, 6              # groups, channels per group (C = G*NI)
CO = 192                   # conv1 out channels
HO, WO = H // 2, W // 2    # 128, 256
HHALF = HO // 2            # 64 output rows per core
NSTEP = 16                 # pipeline steps per core (4 output rows each)
RPB = 2                    # row-pairs per step
ROWSEC = 514               # O(258 incl pad) + E(256)
SECT = (0, 258, 1)         # kj = 0,1,2 section offsets
XROWS = 129                # input rows per core shard incl. pad row
BN_EPS = 1e-5

_PERM = np.array([16 * (c % NI) + (c // NI) for c in range(C)])  # c' -> orig c


def _build(nstep=NSTEP):
    import concourse.bass as bass
    import concourse.tile as tile
    from concourse import bacc, mybir

    nc = bacc.Bacc("TRN2", target_bir_lowering=False, debug=False, num_devices=8)
    f32, bf16 = mybir.dt.float32, mybir.dt.bfloat16

    x_ext = nc.declare_dram_parameter("x", [C, XROWS, ROWSEC], bf16, isOutput=False)
    w1l_ext = nc.declare_dram_parameter("w1l", [C, 18 * C], bf16, isOutput=False)
    w2l_ext = nc.declare_dram_parameter("w2l", [C, 288], bf16, isOutput=False)
    ones_ext = nc.declare_dram_parameter("ones", [128, 512], bf16, isOutput=False)
    bnp_ext = nc.declare_dram_parameter("bnp", [C, 2], f32, isOutput=False)
    out_ext = nc.declare_dram_parameter("out", [C, HHALF, WO], bf16, isOutput=True)

    AP = bass.AP
    mult, add = mybir.AluOpType.mult, mybir.AluOpType.add
    Lrelu, Exp = mybir.ActivationFunctionType.Prelu, mybir.ActivationFunctionType.Exp

    NROW = 4 * RPB + 1         # x slab rows per step (8 new + 1 halo)
    RC = RPB * 512             # step column count (4 rows x 256)

    with tile.TileContext(nc, trace_sim=False) as tc:
        with (
            tc.tile_pool(name="const", bufs=1) as cpool,
            tc.tile_pool(name="xp", bufs=5) as xpool,
            tc.tile_pool(name="yp", bufs=3) as ypool,
            tc.tile_pool(name="expp", bufs=3) as epool,
            tc.tile_pool(name="repp", bufs=3) as rpool,
            tc.tile_pool(name="rcp", bufs=3) as rcpool,
            tc.tile_pool(name="outp", bufs=3) as opool,
            tc.tile_pool(name="py", bufs=2, space="PSUM") as pypool,
            tc.tile_pool(name="pmA", bufs=2, space="PSUM") as pmApool,
            tc.tile_pool(name="pmB", bufs=2, space="PSUM") as pmBpool,
            tc.tile_pool(name="pd", bufs=2, space="PSUM") as pdpool,
        ):
            w1l = cpool.tile([C, 18 * C], bf16)
            nc.sync.dma_start(w1l[:], w1l_ext[:])
            w2l = cpool.tile([C, 288], bf16)
            nc.sync.dma_start(w2l[:], w2l_ext[:])
            ones = cpool.tile([128, 512], bf16)
            nc.sync.dma_start(ones[:], ones_ext[:])
            bnp = cpool.tile([C, 2], f32)
            nc.sync.dma_start(bnp[:], bnp_ext[:])

            def load_x(s):
                x_t = xpool.tile([C, NROW * ROWSEC], bf16)
                nc.sync.dma_start(
                    x_t[:],
                    AP(x_ext[:].tensor, 8 * s * ROWSEC,
                       [[XROWS * ROWSEC, C], [1, NROW * ROWSEC]]),
                )
                return x_t

            def front(s, x_t):
                """conv1/lrelu/conv2/exp/denominators + tap replication."""
                xten, xoff = x_t[:].tensor, x_t[:].offset
                xpart = list(x_t[:].ap[0])
                expA = epool.tile([128, RC], bf16)
                expB = epool.tile([16, RC], bf16)
                psum_d = pdpool.tile([128, 512], mybir.dt.float32)
                rep = rpool.tile([C, 10 * RC], bf16)

                for r in range(RPB):
                    ys = []
                    for ch in range(2):
                        psum_y = pypool.tile([C, 512], mybir.dt.float32)
                        for t9 in range(9):
                            ki, kj = t9 // 3, t9 % 3
                            mv = AP(xten, xoff + (4 * r + ki) * ROWSEC + SECT[kj],
                                    [xpart, [2 * ROWSEC, 2], [1, 256]])
                            nc.tensor.matmul(
                                psum_y[:], w1l[:, (ch * 9 + t9) * C:(ch * 9 + t9 + 1) * C],
                                mv, start=(t9 == 0), stop=(t9 == 8))
                        y = ypool.tile([C, 512], bf16)
                        nc.scalar.activation(y[:], psum_y[:], Lrelu,
                                             bias=bnp[:, ch:ch + 1], scale=1.0, alpha=0.1)
                        ys.append(y)

                    psum_mA = pmApool.tile([128, 512], mybir.dt.float32)
                    nc.tensor.matmul(psum_mA[:], w2l[:, 0:128], ys[0][:], start=True, stop=False)
                    nc.tensor.matmul(psum_mA[:], w2l[:, 128:256], ys[1][:], start=False, stop=True)
                    psum_mB = pmBpool.tile([16, 512], mybir.dt.float32)
                    nc.tensor.matmul(psum_mB[:], w2l[:, 256:272], ys[0][:], start=True, stop=False)
                    nc.tensor.matmul(psum_mB[:], w2l[:, 272:288], ys[1][:], start=False, stop=True)

                    nc.scalar.activation(expA[:, 512 * r:512 * (r + 1)], psum_mA[:], Exp)
                    nc.scalar.activation(expB[:, 512 * r:512 * (r + 1)], psum_mB[:], Exp)

                    nc.tensor.matmul(psum_d[:], ones[:, 128 * r:128 * (r + 1)],
                                     expA[:, 512 * r:512 * (r + 1)],
                                     start=(r == 0), stop=False, skip_group_check=True)
                    nc.tensor.matmul(psum_d[:], ones[0:16, 256 + 128 * r:256 + 128 * (r + 1)],
                                     expB[:, 512 * r:512 * (r + 1)],
                                     start=False, stop=(r == RPB - 1), skip_group_check=True)

                # tap replication: rep[6g+i, t*RC + c] = exp[gt, c]
                eA = expA[:]
                pstA = list(eA.ap[0])[0]
                # issue order alternates source AXI-port parity (taps 0-3
                # read even ports, 4-7 odd) so in-flight reads spread over
                # all 16 ports instead of halves
                for t in (0, 4, 1, 5, 2, 6, 3, 7, 8):
                    if t < 8:
                        bsrc = AP(eA.tensor, eA.offset + t * pstA,
                                  [[8 * pstA, 16], [0, NI], [1, RC]])
                    else:
                        srcB = expB[:]
                        bsrc = AP(srcB.tensor, srcB.offset,
                                  [list(srcB.ap[0]), [0, NI], [1, RC]])
                    nc.sync.dma_start(rep[:, RC * t:RC * (t + 1)], bsrc)

                return {"x_t": x_t, "rep": rep, "psum_d": psum_d}

            def recip_phase(st):
                """1/denominator on DVE + its replication DMAs (sync)."""
                recip32 = rcpool.tile([128, 512], mybir.dt.float32)
                nc.vector.reciprocal_approx_fast(recip32[:], st["psum_d"][:])
                recipbf = rcpool.tile([128, 512], bf16)
                nc.vector.tensor_copy(recipbf[:], recip32[:])
                rep = st["rep"]
                rb = recipbf[:]
                pstR = list(rb.ap[0])[0]
                for r in range(RPB):
                    bsrc = AP(rb.tensor, rb.offset + 4 * r * pstR,
                              [[8 * pstR, 16], [0, NI], [1, 512]])
                    nc.sync.dma_start(
                        rep[:, 9 * RC + 512 * r:9 * RC + 512 * (r + 1)], bsrc)

            def back(st):
                """patch products + tap-sum tree + normalize."""
                x_t, rep = st["x_t"], st["rep"]
                xten, xoff = x_t[:].tensor, x_t[:].offset
                xpart = list(x_t[:].ap[0])
                rten, roff = rep[:].tensor, rep[:].offset
                rpart = list(rep[:].ap[0])
                for r in range(RPB):
                    for kj in range(3):
                        in0 = AP(xten, xoff + 4 * r * ROWSEC + SECT[kj],
                                 [xpart, [ROWSEC, 3], [2 * ROWSEC, 2], [1, 256]])
                        in1 = AP(rten, roff + kj * RC + 512 * r,
                                 [rpart, [3 * RC, 3], [256, 2], [1, 256]])
                        if kj == 2 and r == 0:
                            nc.gpsimd.tensor_tensor(in1, in0, in1, mult)
                        else:
                            nc.vector.tensor_tensor(in1, in0, in1, mult)

                # taps 0-3 += taps 4-7; fold; fold; + tap8; * recip
                nc.vector.tensor_tensor(rep[:, 0:4 * RC], rep[:, 0:4 * RC],
                                        rep[:, 4 * RC:8 * RC], add)
                nc.vector.tensor_tensor(rep[:, 0:2 * RC], rep[:, 0:2 * RC],
                                        rep[:, 2 * RC:4 * RC], add)
                nc.vector.tensor_tensor(rep[:, 0:RC], rep[:, 0:RC],
                                        rep[:, RC:2 * RC], add)
                nc.vector.tensor_tensor(rep[:, 0:RC], rep[:, 0:RC],
                                        rep[:, 8 * RC:9 * RC], add)
                out_t = opool.tile([C, RC], bf16)
                nc.vector.tensor_tensor(out_t[:], rep[:, 0:RC],
                                        rep[:, 9 * RC:10 * RC], mult)
                st["out_t"] = out_t

            def store(s, st):
                nc.scalar.dma_start(
                    AP(out_ext[:].tensor, 4 * s * WO, [[HHALF * WO, C], [1, RC]]),
                    st["out_t"][:])

            # ---- software-pipelined steps ----
            xq = [None] * (nstep + 2)
            xq[0], xq[1] = load_x(0), load_x(1)
            state = [None] * nstep
            for s in range(nstep + 3):
                if s >= 3:
                    store(s - 3, state[s - 3])
                if s + 2 < nstep:
                    xq[s + 2] = load_x(s + 2)
                if s < nstep:
                    state[s] = front(s, xq[s])
                if 2 <= s < nstep + 2:
                    back(state[s - 2])
                if s < nstep:
                    recip_phase(state[s])

    nc.compile()
    return nc


_NC_CACHE = {}


def _get_nc(nstep=NSTEP):
    if nstep not in _NC_CACHE:
        _NC_CACHE[nstep] = _build(nstep)
    return _NC_CACHE[nstep]


def _prep_weights(w1, gamma, beta, w2):
    bnscale = (gamma / np.sqrt(1.0 + BN_EPS)).astype(np.float32)
    w1s = w1.astype(np.float32) * bnscale[:, None, None, None]   # [192,96,3,3]
    # w1l[k, (ch*9+t)*96 + m] = w1s[ch*96+m, PERM[k], ki, kj]
    w1p = w1s[:, _PERM, :, :]                                    # [192,96p,3,3]
    w1l = np.zeros((C, 18 * C), np.float32)
    for ch in range(2):
        for t9 in range(9):
            ki, kj = t9 // 3, t9 % 3
            w1l[:, (ch * 9 + t9) * C:(ch * 9 + t9 + 1) * C] = \
                w1p[ch * C:(ch + 1) * C, :, ki, kj].T
    # w2l: [A0(128) | A1(128) | B0(16) | B1(16)]; A col m=8g+t (port spread),
    # B col g; orig mask channel g*9+t
    w2f = w2.astype(np.float32)[:, :, 0, 0]                      # [144,192]
    w2l = np.zeros((C, 288), np.float32)
    for t in range(8):
        for g in range(G):
            w2l[:, 8 * g + t] = w2f[g * 9 + t, 0:C]
            w2l[:, 128 + 8 * g + t] = w2f[g * 9 + t, C:2 * C]
    for g in range(G):
        w2l[:, 256 + g] = w2f[g * 9 + 8, 0:C]
        w2l[:, 272 + g] = w2f[g * 9 + 8, C:2 * C]
    # ones: denom psum partition for (rp r, group g) is 8g+4r, so the two
    # recip replication DMAs source complementary AXI-port halves.
    # A_r block at cols 128r (rows 8g+t, t<8); B_r at cols 256+128r (rows g).
    ones = np.zeros((128, 512), np.float32)
    for r in range(2):
        for t in range(8):
            for g in range(G):
                ones[8 * g + t, 128 * r + 8 * g + 4 * r] = 1.0
        for g in range(G):
            ones[g, 256 + 128 * r + 8 * g + 4 * r] = 1.0
    bnp = np.stack([beta[0:C], beta[C:2 * C]], axis=1).astype(np.float32)
    return w1l.astype(BF), w2l.astype(BF), ones.astype(BF), bnp


def _prep_x_shard(xb):
    """xb: [C, H, W] f32 already channel-permuted; returns two [C,129,514]
    bf16 shards (top half, bottom half)."""
    shards = []
    for half in range(2):
        if half == 0:
            rows = np.concatenate(
                [np.zeros((C, 1, W), np.float32), xb[:, 0:H // 2, :]], axis=1)
        else:
            rows = xb[:, H // 2 - 1:H, :]
        o = np.concatenate([np.zeros((C, XROWS, 1), np.float32),
                            rows[:, :, 1::2]], axis=2)          # O: 257
        sec = np.concatenate([
            o, np.zeros((C, XROWS, 1), np.float32),              # pad -> 258
            rows[:, :, 0::2]], axis=2)                           # E: 256
        shards.append(sec.astype(BF))
    return shards


def _make_in_maps(inputs):
    w1l, w2l, ones, bnp = _prep_weights(
        inputs["w1"], inputs["gamma"], inputs["beta"], inputs["w2"])
    xp = np.asarray(inputs["x"])[:, _PERM, :, :].astype(np.float32)
    in_maps = []
    for b in range(B):
        halves = _prep_x_shard(xp[b])
        for half in range(2):
            in_maps.append({"x": halves[half], "w1l": w1l, "w2l": w2l,
                            "ones": ones, "bnp": bnp})
    return in_maps


def kernel(x, w1, gamma, beta, w2):
    from concourse.bass_utils import run_bass_kernel_spmd

    nc = _get_nc()
    in_maps = _make_in_maps({"x": x, "w1": w1, "gamma": gamma, "beta": beta, "w2": w2})

    res = run_bass_kernel_spmd(nc, in_maps, core_ids=list(range(8)), trace=False)

    out = np.empty((B, C, HO, WO), np.float32)
    for core in range(8):
        b, half = core // 2, core % 2
        out[b, _PERM, half * HHALF:(half + 1) * HHALF, :] = res.results[core]["out"].astype(np.float32)
    return out
